# revision 60
# baseline (speedup 1.0000x reference)
"""Multi-head attention (B=4, S=2048, D=1024, H=16) on 8 Trainium2 cores.

Sharding: core c -> (batch b = c//2, head-half hh = c%2).  Each core computes
8 heads of one batch: QKV projections with column-sliced weights, attention,
and a partial output projection with row-sliced Wo.  Host sums the two
partial outputs per batch and adds the constant bias terms.

The kernel is ACT(exp)-paced: 256 exp tiles of [128,1024] at ~1.34us each.
Everything else is scheduled to keep the scalar engine saturated:

  - attention runs in 9 groups: (p, qq-pair) for p=0..3 over head-pairs p,
    with the last p split into single-qq groups to shorten the output tail.
  - per key block kb: scores^T for both qq chunks (row-packed head pairs,
    2 concurrent N=512 matmuls per qq), exp on ACT, then (lagged by LAG kb)
    PV pairs (col-packed, concurrent) and Z row sums.
  - Z uses a [128,1] ones stationary -> M=1 matmuls, 4-way col-tiled at
    positions (0,0),(0,32),(0,64),(0,96): one concurrent round covers both
    heads x both qq chunks (~258ns).
  - softmax normalization: 1/Z via fast approx reciprocal on DVE (read
    straight from PSUM), cast to bf16, broadcast via rank-1 bf16 selector
    matmuls, applied to x^T with a DVE mul.
  - projections (V blocks, K/Q chunks), norm broadcasts and output pieces
    drain from a deadline-keyed work queue in the PE slack inside the
    ACT-paced loop.  Attention starts as soon as K(0,0)/Q(0,0..1) are
    projected (~15us) instead of after the full V projection.
  - input DMAs are split in token halves and spread over the sync /scalar /
    gpsimd /vector /tensor queues in deadline order; y is written in bf16
    on sync/gpsimd/vector (never scalar, which must stay free for exp).
"""
import numpy as np
import ml_dtypes

import concourse.tile as tile
from concourse import bacc, mybir
from concourse import bass_utils

F32 = mybir.dt.float32
BF16 = mybir.dt.bfloat16
EXP = mybir.ActivationFunctionType.Exp

B, S, D = 4, 2048, 1024
H = 16
DK = 64
FEAT = 512          # features per core (8 heads)
N_CORES = 8
LAG = 4             # PV/Z lag behind exp, in key blocks

_PROGRAM = None


def _build_program():
    nc = bacc.Bacc("TRN2", target_bir_lowering=False, debug=False,
                   enable_asserts=True, num_devices=N_CORES)

    xq = nc.dram_tensor("xq_t", [D, S], BF16, kind="ExternalInput").ap()
    xk = nc.dram_tensor("xk_t", [D, S], BF16, kind="ExternalInput").ap()
    xv = nc.dram_tensor("xv_t", [D, S], BF16, kind="ExternalInput").ap()
    wq = nc.dram_tensor("wq", [D, FEAT], BF16, kind="ExternalInput").ap()
    wk = nc.dram_tensor("wk", [D, FEAT], BF16, kind="ExternalInput").ap()
    wv = nc.dram_tensor("wv", [D, FEAT], BF16, kind="ExternalInput").ap()
    wo = nc.dram_tensor("wo", [FEAT, D], BF16, kind="ExternalInput").ap()
    bqk = nc.dram_tensor("bqk", [128, 8], F32, kind="ExternalInput").ap()
    sel = nc.dram_tensor("sel", [2, 128], BF16, kind="ExternalInput").ap()
    ones = nc.dram_tensor("ones", [128, 1], BF16, kind="ExternalInput").ap()
    y = nc.dram_tensor("y", [S, D], BF16, kind="ExternalOutput").ap()

    with tile.TileContext(nc) as tc:
        with nc.allow_low_precision(reason="bf16 matmul operand tiles"):
            _emit(nc, tc, xq, xk, xv, wq, wk, wv, wo, bqk, sel, ones, y)
    nc.compile()
    return nc


# attention groups: (p, qqs); last p split for a shorter output tail
GROUPS = [(0, (0, 1)), (0, (2, 3)),
          (1, (0, 1)), (1, (2, 3)),
          (2, (0, 1)), (2, (2, 3)),
          (3, (0, 1)), (3, (2,)), (3, (3,))]
ZROWS = ((0, 64), (32, 96))   # zp rows for qq index 0/1 within a group


def _emit(nc, tc, xq, xk, xv, wq, wk, wv, wo, bqk, sel, ones, y):
    from contextlib import ExitStack
    import bisect
    import itertools

    MM = nc.tensor.matmul

    with ExitStack() as ctx:
        ep = ctx.enter_context

        # ---------- persistent SBUF ----------
        qt_pool = ep(tc.tile_pool(name="qt", bufs=2))
        kt_pool = ep(tc.tile_pool(name="kt", bufs=2))
        v_pool = ep(tc.tile_pool(name="v", bufs=1))
        misc_pool = ep(tc.tile_pool(name="misc", bufs=1))
        xT_pool = ep(tc.tile_pool(name="xT", bufs=1))
        pt_pool = ep(tc.tile_pool(name="pt", bufs=2 * LAG + 2))
        rz_pool = ep(tc.tile_pool(name="rz", bufs=2))
        y_sb_pool = ep(tc.tile_pool(name="ysb", bufs=2))
        xk_pool = ep(tc.tile_pool(name="xk", bufs=1))
        wk_pool = ep(tc.tile_pool(name="wkp", bufs=1))
        xq_pool = ep(tc.tile_pool(name="xqp", bufs=1))
        wq_pool = ep(tc.tile_pool(name="wqp", bufs=1))

        # qt/kt rotate through 2 buffers: head-pair m's tiles are dead once
        # groups 2m/2m+1 finish, and m+2's projections pop much later
        _qt = {}
        _kt = {}

        def qt(m):
            if m not in _qt:
                _qt[m] = qt_pool.tile([128, S], BF16, tag="qt", name=f"qt{m}")
            return _qt[m]

        def kt(m):
            if m not in _kt:
                _kt[m] = kt_pool.tile([128, S], BF16, tag="kt", name=f"kt{m}")
            return _kt[m]
        v_sb = [v_pool.tile([128, FEAT], BF16, tag=f"v{k}", name=f"v{k}") for k in range(16)]
        xT = [xT_pool.tile([128, S], BF16, tag=f"xT{p}", name=f"xT{p}") for p in range(4)]

        bqk_sb = misc_pool.tile([128, 8], F32, tag="bqk")
        bq_sb = bqk_sb[:, 0:4]
        bk_sb = bqk_sb[:, 4:8]
        ones_sb = misc_pool.tile([128, 1], BF16, tag="ones")
        selA_sb = misc_pool.tile([1, 128], BF16, tag="selA")
        selB_sb = misc_pool.tile([1, 128], BF16, tag="selB")
        warm_sb = misc_pool.tile([1, 4], F32, tag="warm")

        xk_sb = [xk_pool.tile([128, S], BF16, tag=f"xk{c}", name=f"xk{c}") for c in range(8)]
        wk_sb = [wk_pool.tile([128, FEAT], BF16, tag=f"wk{c}", name=f"wk{c}") for c in range(8)]
        xq_sb = [xq_pool.tile([128, S], BF16, tag=f"xq{c}", name=f"xq{c}") for c in range(8)]
        wq_sb = [wq_pool.tile([128, FEAT], BF16, tag=f"wq{c}", name=f"wq{c}") for c in range(8)]

        # ---------- input DMAs, deadline-ordered ----------
        # A dma_start occupies its issuing engine queue for roughly the whole
        # transfer (~170GB/s per queue), so the layout is three parallel
        # streams with the scalar(ACT) queue freed early for the exps:
        #   sync:   consts, then xk/xv interleaved in token quarters
        #   scalar: warm-up exp (table preload), wq, xq q0/q1, then free
        #   gpsimd: wk, wv, xq q2/q3, wo
        nc.sync.dma_start(bqk_sb[:], bqk)
        nc.sync.dma_start(selA_sb[:], sel[0:1, :])
        nc.sync.dma_start(selB_sb[:], sel[1:2, :])
        nc.sync.dma_start(ones_sb[:], ones)
        nc.scalar.activation(warm_sb[:], bqk_sb[0:1, 0:4], EXP)
        for c in range(8):
            nc.gpsimd.dma_start(wk_sb[c][:], wk[c * 128:(c + 1) * 128, :])
        for c in range(8):
            nc.sync.dma_start(xk_sb[c][:, 0:512], xk[c * 128:(c + 1) * 128, 0:512])
        for c in range(8):
            nc.scalar.dma_start(wq_sb[c][:], wq[c * 128:(c + 1) * 128, :])
        for c in range(8):
            nc.scalar.dma_start(xq_sb[c][:, 0:512], xq[c * 128:(c + 1) * 128, 0:512])
        for c in range(8):
            nc.sync.dma_start(xq_sb[c][:, 512:1024], xq[c * 128:(c + 1) * 128, 512:1024])

        # ---------- PSUM pools ----------
        st_pool = ep(tc.tile_pool(name="st", bufs=2, space="PSUM"))
        pv_pool = ep(tc.tile_pool(name="pv", bufs=2, space="PSUM"))
        zp_pool = ep(tc.tile_pool(name="zp", bufs=1, space="PSUM"))
        scr_pool = ep(tc.tile_pool(name="scr", bufs=1, space="PSUM"))

        # ---------- work queue ----------
        pend_pe = []
        _tie = itertools.count()

        def push(key, emit):
            bisect.insort(pend_pe, (key, next(_tie), emit))

        def proj_chunk(m, n, w_sb, x_sb, dst, bias_sb, on_act=False):
            def emit():
                psq = scr_pool.tile([128, 512], F32, tag="scr",
                                    name=f"pj{id(dst)}{m}{n}")
                for c in range(8):
                    MM(psq[:],
                       w_sb[c][:, m * 128:(m + 1) * 128],
                       x_sb[c][:, n * 512:(n + 1) * 512],
                       start=(c == 0), stop=(c == 7))
                if on_act:
                    nc.scalar.activation(
                        dst(m)[:, n * 512:(n + 1) * 512], psq[:],
                        mybir.ActivationFunctionType.Identity,
                        bias=bias_sb[:, m:m + 1])
                else:
                    nc.vector.tensor_scalar_add(
                        dst(m)[:, n * 512:(n + 1) * 512], psq[:],
                        bias_sb[:, m:m + 1])
            return emit

        def k_chunk(m, n, on_act=False):
            return proj_chunk(m, n, wk_sb, xk_sb, kt, bk_sb, on_act)

        def q_chunk(m, n, on_act=False):
            return proj_chunk(m, n, wq_sb, xq_sb, qt, bq_sb, on_act)

        wo_sb = [None] * 4  # filled after the xv scope closes

        _dma_rr = itertools.count()

        def out_piece(qb, fo, pool_tag=None):
            def emit():
                pool, tag = pool_tag or (scr_pool, "scr")
                yp = pool.tile([128, 512], F32, tag=tag,
                               name=f"yp{qb}{fo}")
                for pp in range(4):
                    MM(yp[:],
                       xT[pp][:, qb * 128:(qb + 1) * 128],
                       wo_sb[pp][:, fo * 512:(fo + 1) * 512],
                       start=(pp == 0), stop=(pp == 3))
                ysb = y_sb_pool.tile([128, 512], BF16, tag="ysb")
                nc.vector.tensor_copy(ysb[:], yp[:])
                eng = (nc.sync, nc.gpsimd)[next(_dma_rr) % 2]
                eng.dma_start(
                    y[qb * 128:(qb + 1) * 128, fo * 512:(fo + 1) * 512],
                    ysb[:])
            return emit

        def norm_piece(p, qq, rzbA, rzbB, now_key):
            def emit():
                bc = scr_pool.tile([128, 512], F32, tag="scr",
                                   name=f"bc{p}{qq}")
                MM(bc[:], selA_sb[:], rzbA[:], start=True, stop=False)
                MM(bc[:], selB_sb[:], rzbB[:], start=False, stop=True)
                nc.vector.tensor_mul(xT[p][:, qq * 512:(qq + 1) * 512],
                                     xT[p][:, qq * 512:(qq + 1) * 512],
                                     bc[:])
                if p == 3:
                    # tail pieces (qq>=2) rotate scratch across the freed
                    # attention PSUM banks -> dense back-to-back matmuls
                    rot = ([(scr_pool, "scr")] if qq < 2 else
                           [(scr_pool, "scr"), (st_pool, "st"),
                            (pv_pool, "pv"), (zp_pool, "zp")])
                    for j, (qb, fo) in enumerate(
                            (qb, fo) for qb in range(4 * qq, 4 * qq + 4)
                            for fo in range(2)):
                        push(now_key + 2 + 0.5 * j,
                             out_piece(qb, fo, rot[j % len(rot)]))
            return emit

        # ---------- attention machinery ----------
        pend_pvz = []   # (p, kb, [(i, qq, pt)], pvs, zp, is_last, qqs)

        def emit_pvz(p, kb, parts, pvs, zp):
            for (i, qq, pt) in parts:
                MM(pvs[i][0:64, :],
                   v_sb[kb][:, p * 128:p * 128 + 64],
                   pt[:, 0:512],
                   tile_position=(0, 0),
                   start=(kb == 0), stop=(kb == 15))
                MM(pvs[i][64:128, :],
                   v_sb[kb][:, p * 128 + 64:p * 128 + 128],
                   pt[:, 512:1024],
                   tile_position=(0, 64),
                   start=(kb == 0), stop=(kb == 15),
                   skip_group_check=True)
            for (i, qq, pt) in parts:
                r0, r1 = ZROWS[i]
                MM(zp[r0:r0 + 1, :],
                   ones_sb[:],
                   pt[:, 0:512],
                   tile_position=(0, r0),
                   start=(kb == 0), stop=(kb == 15),
                   skip_group_check=True)
                MM(zp[r1:r1 + 1, :],
                   ones_sb[:],
                   pt[:, 512:1024],
                   tile_position=(0, r1),
                   start=(kb == 0), stop=(kb == 15),
                   skip_group_check=True)

        def finish_group(p, qqs, pvs, zp, now_key):
            # Ordered to release the PSUM WARs fastest: xT copies free the
            # pv banks, then z-row copies free the zp bank, then the
            # reciprocal/cast chain feeds the (delayed) norm broadcasts.
            for i, qq in enumerate(qqs):
                nc.vector.tensor_copy(xT[p][:, qq * 512:(qq + 1) * 512],
                                      pvs[i][:])
            zrows = []
            for i, qq in enumerate(qqs):
                r0, r1 = ZROWS[i]
                zA = rz_pool.tile([1, 512], F32, tag="zfa")
                zB = rz_pool.tile([1, 512], F32, tag="zfb")
                nc.vector.tensor_copy(zA[:], zp[r0:r0 + 1, :])
                nc.vector.tensor_copy(zB[:], zp[r1:r1 + 1, :])
                zrows.append((zA, zB))
            for i, qq in enumerate(qqs):
                zA, zB = zrows[i]
                rzbA = rz_pool.tile([1, 512], BF16, tag="rzba")
                rzbB = rz_pool.tile([1, 512], BF16, tag="rzbb")
                rzA = rz_pool.tile([1, 512], F32, tag="rzf", bufs=1)
                nc.vector.reciprocal_approx_fast(rzA[:], zA[:])
                nc.vector.tensor_copy(rzbA[:], rzA[:])
                rzB = rz_pool.tile([1, 512], F32, tag="rzf", bufs=1)
                nc.vector.reciprocal_approx_fast(rzB[:], zB[:])
                nc.vector.tensor_copy(rzbB[:], rzB[:])
                push(now_key + 5 + i, norm_piece(p, qq, rzbA, rzbB, now_key))

        def drain_pvz(n_keep, now_key):
            while len(pend_pvz) > n_keep:
                (p_, kb_, parts, pvs_, zp_, last, qqs_) = pend_pvz.pop(0)
                emit_pvz(p_, kb_, parts, pvs_, zp_)
                if last:
                    finish_group(p_, qqs_, pvs_, zp_, now_key)

        def pops(t, budget, horizon):
            while pend_pe and pend_pe[0][0] <= t + 1:
                pend_pe.pop(0)[2]()
            n = 0
            while n < budget and pend_pe and pend_pe[0][0] <= t + horizon:
                pend_pe.pop(0)[2]()
                n += 1

        def emit_group(g_t, p, qqs, budget, horizon):
            pvs = [pv_pool.tile([128, 512], F32, tag="pv",
                                name=f"pv{p}{qq}") for qq in qqs]
            zp = zp_pool.tile([128, 512], F32, tag="zp", name=f"zp{p}{qqs[0]}")
            for kb in range(16):
                t = g_t + kb
                parts = []
                for i, qq in enumerate(qqs):
                    st = st_pool.tile([128, 1024], F32, tag="st")
                    MM(st[:, 0:512],
                       kt(p)[0:64, kb * 128:(kb + 1) * 128],
                       qt(p)[0:64, qq * 512:(qq + 1) * 512],
                       start=True, stop=True)
                    MM(st[:, 512:1024],
                       kt(p)[64:128, kb * 128:(kb + 1) * 128],
                       qt(p)[64:128, qq * 512:(qq + 1) * 512],
                       start=True, stop=True)
                    pt = pt_pool.tile([128, 1024], BF16, tag="pt")
                    nc.scalar.activation(pt[:], st[:], EXP)
                    parts.append((i, qq, pt))
                pend_pvz.append((p, kb, parts, pvs, zp, kb == 15, qqs))
                # split the pops around the PV/Z drain: the attention matmuls
                # (no scratch-bank dependency) fill the scr WAR window between
                # consecutive queue items
                pops(t, 1, horizon)
                drain_pvz(2 if kb >= 14 else LAG, t)
                if budget > 1:
                    pops(t, budget - 1, horizon)
            # group end: release any deadline-gated items, then drain the
            # PV/Z tail so the finish chain starts at the boundary; pop a
            # couple of filler items to keep the PE array warm (HAM) while
            # the DVE finish chain runs
            pops(g_t + 17, 0, 0)
            drain_pvz(0, g_t + 16)
            pops(g_t + 16, 2, 40)

        # ---------- head phase + group 0 (inside the xv/wv scope) ----------
        with tc.tile_pool(name="xv", bufs=1) as xv_pool, \
             tc.tile_pool(name="wvp", bufs=1) as wv_pool:
            xv_sb = [xv_pool.tile([128, S], BF16, tag=f"xv{c}", name=f"xv{c}")
                     for c in range(8)]
            wv_sb = [wv_pool.tile([128, FEAT], BF16, tag=f"wv{c}", name=f"wv{c}")
                     for c in range(8)]
            # remaining bulk in deadline order:
            #   sync:   xv q0, xk q2, xv q2, xq q2, xq q3
            #   gpsimd: wv, xk q1, xv q1, xk q3, xv q3 (then wo)
            for c in range(8):
                nc.gpsimd.dma_start(wv_sb[c][:], wv[c * 128:(c + 1) * 128, :])
            for c in range(8):
                nc.sync.dma_start(xv_sb[c][:, 0:512], xv[c * 128:(c + 1) * 128, 0:512])
            for c in range(8):
                nc.gpsimd.dma_start(xk_sb[c][:, 512:1024], xk[c * 128:(c + 1) * 128, 512:1024])
            for c in range(8):
                nc.sync.dma_start(xk_sb[c][:, 1024:1536], xk[c * 128:(c + 1) * 128, 1024:1536])
            for c in range(8):
                nc.gpsimd.dma_start(xv_sb[c][:, 512:1024], xv[c * 128:(c + 1) * 128, 512:1024])
            for c in range(8):
                nc.sync.dma_start(xv_sb[c][:, 1024:1536], xv[c * 128:(c + 1) * 128, 1024:1536])
            for c in range(8):
                nc.gpsimd.dma_start(xk_sb[c][:, 1536:2048], xk[c * 128:(c + 1) * 128, 1536:2048])
            for c in range(8):
                nc.sync.dma_start(xq_sb[c][:, 1024:1536], xq[c * 128:(c + 1) * 128, 1024:1536])
            for c in range(8):
                nc.gpsimd.dma_start(xv_sb[c][:, 1536:2048], xv[c * 128:(c + 1) * 128, 1536:2048])
            for c in range(8):
                nc.sync.dma_start(xq_sb[c][:, 1536:2048], xq[c * 128:(c + 1) * 128, 1536:2048])

            def v_block(j):
                def emit():
                    vps = scr_pool.tile([128, 512], F32, tag="scr",
                                        name=f"vps{j}")
                    for c in range(8):
                        MM(vps[:],
                           xv_sb[c][:, j * 128:(j + 1) * 128],
                           wv_sb[c][:],
                           start=(c == 0), stop=(c == 7))
                    nc.vector.tensor_copy(v_sb[j][:], vps[:])
                return emit

            # pre-attention: just enough projections to start group 0
            k_chunk(0, 0)()
            q_chunk(0, 0)()
            q_chunk(0, 1)()

            # queue: V blocks + remaining K(0,*) during group 0; later K/Q
            # staggered ~1 item per kb ahead of their group deadlines
            for j in range(16):
                push(j + 3, v_block(j))
            push(3.5, k_chunk(0, 1))
            push(7.5, k_chunk(0, 2))
            push(11.2, k_chunk(0, 3))
            push(12.5, q_chunk(0, 2))
            push(14.5, q_chunk(0, 3))
            for m in range(1, 4):
                g_k = 2 * m * 16
                for n in range(4):
                    push(g_k + 4 * n - 6, k_chunk(m, n))
                for n in range(4):
                    push(g_k + 16 * (n // 2) - 8 + 4 * (n % 2), q_chunk(m, n))

            emit_group(0, 0, (0, 1), budget=1, horizon=4)

        # xv/wv SBUF freed; wo loads into that range
        wo_pool = ep(tc.tile_pool(name="wo", bufs=1))
        for p_ in range(4):
            wo_sb[p_] = wo_pool.tile([128, D], BF16, tag=f"wo{p_}", name=f"wo{p_}")
            nc.gpsimd.dma_start(wo_sb[p_][:], wo[p_ * 128:(p_ + 1) * 128, :])

        g_t = 16
        for (p, qqs) in GROUPS[1:]:
            budget = 2 if p == 3 else 1
            horizon = 48 if p == 3 else 20
            emit_group(g_t, p, qqs, budget=budget, horizon=horizon)
            g_t += 16

        # ---------- tail ----------
        drain_pvz(0, g_t)
        while pend_pe:
            pend_pe.pop(0)[2]()


def get_program():
    global _PROGRAM
    if _PROGRAM is None:
        _PROGRAM = _build_program()
    return _PROGRAM


def make_in_maps(Q_in, K_in, V_in, Wq, bq, Wk, bk, Wv, bv, Wo, bo):
    """Shard full inputs into 8 per-core input maps (bf16 pre-cast on host)."""
    scale = np.float32(1.0 / np.sqrt(DK))
    sel = np.zeros((2, 128), np.float32)
    sel[0, 0:64] = 1.0
    sel[1, 64:128] = 1.0
    sel = sel.astype(ml_dtypes.bfloat16)
    ones = np.ones((128, 1), ml_dtypes.bfloat16)

    def b16(a):
        return np.ascontiguousarray(np.asarray(a, np.float32).astype(ml_dtypes.bfloat16))

    xt = {}
    for b in range(B):
        xt[b] = (b16(np.asarray(Q_in[b], np.float32).T),
                 b16(np.asarray(K_in[b], np.float32).T),
                 b16(np.asarray(V_in[b], np.float32).T))

    in_maps = []
    for c in range(N_CORES):
        b, hh = c // 2, c % 2
        sl = slice(hh * FEAT, (hh + 1) * FEAT)
        in_maps.append({
            "xq_t": xt[b][0],
            "xk_t": xt[b][1],
            "xv_t": xt[b][2],
            "wq": b16(np.asarray(Wq, np.float32)[:, sl]),
            "wk": b16(np.asarray(Wk, np.float32)[:, sl] * scale),
            "wv": b16(np.asarray(Wv, np.float32)[:, sl]),
            "wo": b16(np.asarray(Wo, np.float32)[sl, :]),
            "bqk": np.ascontiguousarray(np.concatenate(
                [np.asarray(bq, np.float32)[sl].reshape(4, 128).T,
                 np.asarray(bk, np.float32)[sl].reshape(4, 128).T * scale],
                axis=1)),
            "sel": sel,
            "ones": ones,
        })
    return in_maps


def gather_output(results, Wo, bv, bo):
    """Combine per-core partial outputs into the full [B, S, D] output."""
    const = (np.asarray(bv, np.float32) @ np.asarray(Wo, np.float32)
             + np.asarray(bo, np.float32))
    out = np.empty((B, S, D), np.float32)
    for b in range(B):
        out[b] = (np.asarray(results[2 * b]["y"], np.float32)
                  + np.asarray(results[2 * b + 1]["y"], np.float32) + const)
    return out


def kernel(Q_in, K_in, V_in, Wq, bq, Wk, bk, Wv, bv, Wo, bo):
    nc = get_program()
    in_maps = make_in_maps(Q_in, K_in, V_in, Wq, bq, Wk, bk, Wv, bv, Wo, bo)
    res = bass_utils.run_bass_kernel_spmd(nc, in_maps, core_ids=list(range(N_CORES)))
    return gather_output(res.results, Wo, bv, bo)


# revision 62
# speedup vs baseline: 1.0039x; 1.0039x over previous
"""Multi-head attention (B=4, S=2048, D=1024, H=16) on 8 Trainium2 cores.

Sharding: core c -> (batch b = c//2, head-half hh = c%2).  Each core computes
8 heads of one batch: QKV projections with column-sliced weights, attention,
and a partial output projection with row-sliced Wo.  Host sums the two
partial outputs per batch and adds the constant bias terms.

The kernel is ACT(exp)-paced: 256 exp tiles of [128,1024] at ~1.34us each.
Everything else is scheduled to keep the scalar engine saturated:

  - attention runs in 9 groups: (p, qq-pair) for p=0..3 over head-pairs p,
    with the last p split into single-qq groups to shorten the output tail.
  - per key block kb: scores^T for both qq chunks (row-packed head pairs,
    2 concurrent N=512 matmuls per qq), exp on ACT, then (lagged by LAG kb)
    PV pairs (col-packed, concurrent) and Z row sums.
  - Z uses a [128,1] ones stationary -> M=1 matmuls, 4-way col-tiled at
    positions (0,0),(0,32),(0,64),(0,96): one concurrent round covers both
    heads x both qq chunks (~258ns).
  - softmax normalization: 1/Z via fast approx reciprocal on DVE (read
    straight from PSUM), cast to bf16, broadcast via rank-1 bf16 selector
    matmuls, applied to x^T with a DVE mul.
  - projections (V blocks, K/Q chunks), norm broadcasts and output pieces
    drain from a deadline-keyed work queue in the PE slack inside the
    ACT-paced loop.  Attention starts as soon as K(0,0)/Q(0,0..1) are
    projected (~15us) instead of after the full V projection.
  - input DMAs are split in token halves and spread over the sync /scalar /
    gpsimd /vector /tensor queues in deadline order; y is written in bf16
    on sync/gpsimd/vector (never scalar, which must stay free for exp).
"""
import numpy as np
import ml_dtypes

import concourse.tile as tile
from concourse import bacc, mybir
from concourse import bass_utils

F32 = mybir.dt.float32
BF16 = mybir.dt.bfloat16
EXP = mybir.ActivationFunctionType.Exp

B, S, D = 4, 2048, 1024
H = 16
DK = 64
FEAT = 512          # features per core (8 heads)
N_CORES = 8
LAG = 2             # PV/Z lag behind exp, in key blocks

_PROGRAM = None


def _build_program():
    nc = bacc.Bacc("TRN2", target_bir_lowering=False, debug=False,
                   enable_asserts=True, num_devices=N_CORES)

    xq = nc.dram_tensor("xq_t", [D, S], BF16, kind="ExternalInput").ap()
    xk = nc.dram_tensor("xk_t", [D, S], BF16, kind="ExternalInput").ap()
    xv = nc.dram_tensor("xv_t", [D, S], BF16, kind="ExternalInput").ap()
    wq = nc.dram_tensor("wq", [D, FEAT], BF16, kind="ExternalInput").ap()
    wk = nc.dram_tensor("wk", [D, FEAT], BF16, kind="ExternalInput").ap()
    wv = nc.dram_tensor("wv", [D, FEAT], BF16, kind="ExternalInput").ap()
    wo = nc.dram_tensor("wo", [FEAT, D], BF16, kind="ExternalInput").ap()
    bqk = nc.dram_tensor("bqk", [128, 8], F32, kind="ExternalInput").ap()
    sel = nc.dram_tensor("sel", [2, 128], BF16, kind="ExternalInput").ap()
    ones = nc.dram_tensor("ones", [128, 1], BF16, kind="ExternalInput").ap()
    y = nc.dram_tensor("y", [S, D], BF16, kind="ExternalOutput").ap()

    with tile.TileContext(nc) as tc:
        with nc.allow_low_precision(reason="bf16 matmul operand tiles"):
            _emit(nc, tc, xq, xk, xv, wq, wk, wv, wo, bqk, sel, ones, y)
    nc.compile()
    return nc


# attention groups: (p, qqs); last p split for a shorter output tail
GROUPS = [(0, (0, 1)), (0, (2, 3)),
          (1, (0, 1)), (1, (2, 3)),
          (2, (0, 1)), (2, (2, 3)),
          (3, (0, 1)), (3, (2,)), (3, (3,))]
ZROWS = ((0, 64), (32, 96))   # zp rows for qq index 0/1 within a group


def _emit(nc, tc, xq, xk, xv, wq, wk, wv, wo, bqk, sel, ones, y):
    from contextlib import ExitStack
    import bisect
    import itertools

    MM = nc.tensor.matmul

    with ExitStack() as ctx:
        ep = ctx.enter_context

        # ---------- persistent SBUF ----------
        qt_pool = ep(tc.tile_pool(name="qt", bufs=2))
        kt_pool = ep(tc.tile_pool(name="kt", bufs=2))
        v_pool = ep(tc.tile_pool(name="v", bufs=1))
        misc_pool = ep(tc.tile_pool(name="misc", bufs=1))
        xT_pool = ep(tc.tile_pool(name="xT", bufs=1))
        pt_pool = ep(tc.tile_pool(name="pt", bufs=2 * LAG + 2))
        rz_pool = ep(tc.tile_pool(name="rz", bufs=2))
        y_sb_pool = ep(tc.tile_pool(name="ysb", bufs=2))
        xk_pool = ep(tc.tile_pool(name="xk", bufs=1))
        wk_pool = ep(tc.tile_pool(name="wkp", bufs=1))
        xq_pool = ep(tc.tile_pool(name="xqp", bufs=1))
        wq_pool = ep(tc.tile_pool(name="wqp", bufs=1))

        # qt/kt rotate through 2 buffers: head-pair m's tiles are dead once
        # groups 2m/2m+1 finish, and m+2's projections pop much later
        _qt = {}
        _kt = {}

        def qt(m):
            if m not in _qt:
                _qt[m] = qt_pool.tile([128, S], BF16, tag="qt", name=f"qt{m}")
            return _qt[m]

        def kt(m):
            if m not in _kt:
                _kt[m] = kt_pool.tile([128, S], BF16, tag="kt", name=f"kt{m}")
            return _kt[m]
        v_sb = [v_pool.tile([128, FEAT], BF16, tag=f"v{k}", name=f"v{k}") for k in range(16)]
        xT = [xT_pool.tile([128, S], BF16, tag=f"xT{p}", name=f"xT{p}") for p in range(4)]

        bqk_sb = misc_pool.tile([128, 8], F32, tag="bqk")
        bq_sb = bqk_sb[:, 0:4]
        bk_sb = bqk_sb[:, 4:8]
        ones_sb = misc_pool.tile([128, 1], BF16, tag="ones")
        selA_sb = misc_pool.tile([1, 128], BF16, tag="selA")
        selB_sb = misc_pool.tile([1, 128], BF16, tag="selB")
        warm_sb = misc_pool.tile([1, 4], F32, tag="warm")

        xk_sb = [xk_pool.tile([128, S], BF16, tag=f"xk{c}", name=f"xk{c}") for c in range(8)]
        wk_sb = [wk_pool.tile([128, FEAT], BF16, tag=f"wk{c}", name=f"wk{c}") for c in range(8)]
        xq_sb = [xq_pool.tile([128, S], BF16, tag=f"xq{c}", name=f"xq{c}") for c in range(8)]
        wq_sb = [wq_pool.tile([128, FEAT], BF16, tag=f"wq{c}", name=f"wq{c}") for c in range(8)]

        # ---------- input DMAs, deadline-ordered ----------
        # A dma_start occupies its issuing engine queue for roughly the whole
        # transfer (~170GB/s per queue), so the layout is three parallel
        # streams with the scalar(ACT) queue freed early for the exps:
        #   sync:   consts, then xk/xv interleaved in token quarters
        #   scalar: warm-up exp (table preload), wq, xq q0/q1, then free
        #   gpsimd: wk, wv, xq q2/q3, wo
        nc.sync.dma_start(bqk_sb[:], bqk)
        nc.sync.dma_start(selA_sb[:], sel[0:1, :])
        nc.sync.dma_start(selB_sb[:], sel[1:2, :])
        nc.sync.dma_start(ones_sb[:], ones)
        nc.scalar.activation(warm_sb[:], bqk_sb[0:1, 0:4], EXP)
        for c in range(8):
            nc.gpsimd.dma_start(wk_sb[c][:], wk[c * 128:(c + 1) * 128, :])
        for c in range(8):
            nc.sync.dma_start(xk_sb[c][:, 0:512], xk[c * 128:(c + 1) * 128, 0:512])
        for c in range(8):
            nc.scalar.dma_start(wq_sb[c][:], wq[c * 128:(c + 1) * 128, :])
        for c in range(8):
            nc.scalar.dma_start(xq_sb[c][:, 0:512], xq[c * 128:(c + 1) * 128, 0:512])
        for c in range(8):
            nc.sync.dma_start(xq_sb[c][:, 512:1024], xq[c * 128:(c + 1) * 128, 512:1024])

        # ---------- PSUM pools ----------
        st_pool = ep(tc.tile_pool(name="st", bufs=2, space="PSUM"))
        pv_pool = ep(tc.tile_pool(name="pv", bufs=2, space="PSUM"))
        zp_pool = ep(tc.tile_pool(name="zp", bufs=1, space="PSUM"))
        scr_pool = ep(tc.tile_pool(name="scr", bufs=1, space="PSUM"))

        # ---------- work queue ----------
        pend_pe = []
        _tie = itertools.count()

        def push(key, emit):
            bisect.insort(pend_pe, (key, next(_tie), emit))

        def proj_chunk(m, n, w_sb, x_sb, dst, bias_sb, on_act=False):
            def emit():
                psq = scr_pool.tile([128, 512], F32, tag="scr",
                                    name=f"pj{id(dst)}{m}{n}")
                for c in range(8):
                    MM(psq[:],
                       w_sb[c][:, m * 128:(m + 1) * 128],
                       x_sb[c][:, n * 512:(n + 1) * 512],
                       start=(c == 0), stop=(c == 7))
                if on_act:
                    nc.scalar.activation(
                        dst(m)[:, n * 512:(n + 1) * 512], psq[:],
                        mybir.ActivationFunctionType.Identity,
                        bias=bias_sb[:, m:m + 1])
                else:
                    nc.vector.tensor_scalar_add(
                        dst(m)[:, n * 512:(n + 1) * 512], psq[:],
                        bias_sb[:, m:m + 1])
            return emit

        def k_chunk(m, n, on_act=False):
            return proj_chunk(m, n, wk_sb, xk_sb, kt, bk_sb, on_act)

        def q_chunk(m, n, on_act=False):
            return proj_chunk(m, n, wq_sb, xq_sb, qt, bq_sb, on_act)

        wo_sb = [None] * 4  # filled after the xv scope closes

        _dma_rr = itertools.count()

        def out_piece(qb, fo, pool_tag=None):
            def emit():
                pool, tag = pool_tag or (scr_pool, "scr")
                yp = pool.tile([128, 512], F32, tag=tag,
                               name=f"yp{qb}{fo}")
                for pp in range(4):
                    MM(yp[:],
                       xT[pp][:, qb * 128:(qb + 1) * 128],
                       wo_sb[pp][:, fo * 512:(fo + 1) * 512],
                       start=(pp == 0), stop=(pp == 3))
                ysb = y_sb_pool.tile([128, 512], BF16, tag="ysb")
                nc.vector.tensor_copy(ysb[:], yp[:])
                eng = (nc.sync, nc.gpsimd)[next(_dma_rr) % 2]
                eng.dma_start(
                    y[qb * 128:(qb + 1) * 128, fo * 512:(fo + 1) * 512],
                    ysb[:])
            return emit

        def norm_piece(p, qq, rzbA, rzbB, now_key):
            def emit():
                bc = scr_pool.tile([128, 512], F32, tag="scr",
                                   name=f"bc{p}{qq}")
                MM(bc[:], selA_sb[:], rzbA[:], start=True, stop=False)
                MM(bc[:], selB_sb[:], rzbB[:], start=False, stop=True)
                nc.vector.tensor_mul(xT[p][:, qq * 512:(qq + 1) * 512],
                                     xT[p][:, qq * 512:(qq + 1) * 512],
                                     bc[:])
                if p == 3:
                    # tail pieces (qq>=2) rotate scratch across the freed
                    # attention PSUM banks -> dense back-to-back matmuls
                    rot = ([(scr_pool, "scr")] if qq < 2 else
                           [(scr_pool, "scr"), (st_pool, "st"),
                            (pv_pool, "pv"), (zp_pool, "zp")])
                    for j, (qb, fo) in enumerate(
                            (qb, fo) for qb in range(4 * qq, 4 * qq + 4)
                            for fo in range(2)):
                        push(now_key + 2 + 0.5 * j,
                             out_piece(qb, fo, rot[j % len(rot)]))
            return emit

        # ---------- attention machinery ----------
        pend_pvz = []   # (p, kb, [(i, qq, pt)], pvs, zp, is_last, qqs)

        def emit_pvz(p, kb, parts, pvs, zp):
            for (i, qq, pt) in parts:
                MM(pvs[i][0:64, :],
                   v_sb[kb][:, p * 128:p * 128 + 64],
                   pt[:, 0:512],
                   tile_position=(0, 0),
                   start=(kb == 0), stop=(kb == 15))
                MM(pvs[i][64:128, :],
                   v_sb[kb][:, p * 128 + 64:p * 128 + 128],
                   pt[:, 512:1024],
                   tile_position=(0, 64),
                   start=(kb == 0), stop=(kb == 15),
                   skip_group_check=True)
            for (i, qq, pt) in parts:
                r0, r1 = ZROWS[i]
                MM(zp[r0:r0 + 1, :],
                   ones_sb[:],
                   pt[:, 0:512],
                   tile_position=(0, r0),
                   start=(kb == 0), stop=(kb == 15),
                   skip_group_check=True)
                MM(zp[r1:r1 + 1, :],
                   ones_sb[:],
                   pt[:, 512:1024],
                   tile_position=(0, r1),
                   start=(kb == 0), stop=(kb == 15),
                   skip_group_check=True)

        def finish_group(p, qqs, pvs, zp, now_key):
            # Ordered to release the PSUM WARs fastest: xT copies free the
            # pv banks, then z-row copies free the zp bank, then the
            # reciprocal/cast chain feeds the (delayed) norm broadcasts.
            for i, qq in enumerate(qqs):
                nc.vector.tensor_copy(xT[p][:, qq * 512:(qq + 1) * 512],
                                      pvs[i][:])
            zrows = []
            for i, qq in enumerate(qqs):
                r0, r1 = ZROWS[i]
                zA = rz_pool.tile([1, 512], F32, tag="zfa")
                zB = rz_pool.tile([1, 512], F32, tag="zfb")
                nc.vector.tensor_copy(zA[:], zp[r0:r0 + 1, :])
                nc.vector.tensor_copy(zB[:], zp[r1:r1 + 1, :])
                zrows.append((zA, zB))
            for i, qq in enumerate(qqs):
                zA, zB = zrows[i]
                rzbA = rz_pool.tile([1, 512], BF16, tag="rzba")
                rzbB = rz_pool.tile([1, 512], BF16, tag="rzbb")
                rzA = rz_pool.tile([1, 512], F32, tag="rzf", bufs=1)
                nc.vector.reciprocal_approx_fast(rzA[:], zA[:])
                nc.vector.tensor_copy(rzbA[:], rzA[:])
                rzB = rz_pool.tile([1, 512], F32, tag="rzf", bufs=1)
                nc.vector.reciprocal_approx_fast(rzB[:], zB[:])
                nc.vector.tensor_copy(rzbB[:], rzB[:])
                push(now_key + 5 + i, norm_piece(p, qq, rzbA, rzbB, now_key))

        def drain_pvz(n_keep, now_key):
            while len(pend_pvz) > n_keep:
                (p_, kb_, parts, pvs_, zp_, last, qqs_) = pend_pvz.pop(0)
                emit_pvz(p_, kb_, parts, pvs_, zp_)
                if last:
                    finish_group(p_, qqs_, pvs_, zp_, now_key)

        def pops(t, budget, horizon):
            while pend_pe and pend_pe[0][0] <= t + 1:
                pend_pe.pop(0)[2]()
            n = 0
            while n < budget and pend_pe and pend_pe[0][0] <= t + horizon:
                pend_pe.pop(0)[2]()
                n += 1

        def emit_group(g_t, p, qqs, budget, horizon):
            pvs = [pv_pool.tile([128, 512], F32, tag="pv",
                                name=f"pv{p}{qq}") for qq in qqs]
            zp = zp_pool.tile([128, 512], F32, tag="zp", name=f"zp{p}{qqs[0]}")
            for kb in range(16):
                t = g_t + kb
                parts = []
                for i, qq in enumerate(qqs):
                    st = st_pool.tile([128, 1024], F32, tag="st")
                    MM(st[:, 0:512],
                       kt(p)[0:64, kb * 128:(kb + 1) * 128],
                       qt(p)[0:64, qq * 512:(qq + 1) * 512],
                       start=True, stop=True)
                    MM(st[:, 512:1024],
                       kt(p)[64:128, kb * 128:(kb + 1) * 128],
                       qt(p)[64:128, qq * 512:(qq + 1) * 512],
                       start=True, stop=True)
                    pt = pt_pool.tile([128, 1024], BF16, tag="pt")
                    nc.scalar.activation(pt[:], st[:], EXP)
                    parts.append((i, qq, pt))
                pend_pvz.append((p, kb, parts, pvs, zp, kb == 15, qqs))
                # split the pops around the PV/Z drain: the attention matmuls
                # (no scratch-bank dependency) fill the scr WAR window between
                # consecutive queue items
                pops(t, 1, horizon)
                drain_pvz(LAG, t)
                if budget > 1:
                    pops(t, budget - 1, horizon)
            # group end: release any deadline-gated items, then drain the
            # PV/Z tail so the finish chain starts at the boundary; pop a
            # couple of filler items to keep the PE array warm (HAM) while
            # the DVE finish chain runs
            pops(g_t + 17, 0, 0)
            drain_pvz(0, g_t + 16)
            pops(g_t + 16, 2, 40)

        # ---------- head phase + group 0 (inside the xv/wv scope) ----------
        with tc.tile_pool(name="xv", bufs=1) as xv_pool, \
             tc.tile_pool(name="wvp", bufs=1) as wv_pool:
            xv_sb = [xv_pool.tile([128, S], BF16, tag=f"xv{c}", name=f"xv{c}")
                     for c in range(8)]
            wv_sb = [wv_pool.tile([128, FEAT], BF16, tag=f"wv{c}", name=f"wv{c}")
                     for c in range(8)]
            # remaining bulk in deadline order:
            #   sync:   xv q0, xk q2, xv q2, xq q2, xq q3
            #   gpsimd: wv, xk q1, xv q1, xk q3, xv q3 (then wo)
            for c in range(8):
                nc.gpsimd.dma_start(wv_sb[c][:], wv[c * 128:(c + 1) * 128, :])
            for c in range(8):
                nc.sync.dma_start(xv_sb[c][:, 0:512], xv[c * 128:(c + 1) * 128, 0:512])
            for c in range(8):
                nc.gpsimd.dma_start(xk_sb[c][:, 512:1024], xk[c * 128:(c + 1) * 128, 512:1024])
            for c in range(8):
                nc.sync.dma_start(xk_sb[c][:, 1024:1536], xk[c * 128:(c + 1) * 128, 1024:1536])
            for c in range(8):
                nc.gpsimd.dma_start(xv_sb[c][:, 512:1024], xv[c * 128:(c + 1) * 128, 512:1024])
            for c in range(8):
                nc.sync.dma_start(xv_sb[c][:, 1024:1536], xv[c * 128:(c + 1) * 128, 1024:1536])
            for c in range(8):
                nc.gpsimd.dma_start(xk_sb[c][:, 1536:2048], xk[c * 128:(c + 1) * 128, 1536:2048])
            for c in range(8):
                nc.sync.dma_start(xq_sb[c][:, 1024:1536], xq[c * 128:(c + 1) * 128, 1024:1536])
            for c in range(8):
                nc.gpsimd.dma_start(xv_sb[c][:, 1536:2048], xv[c * 128:(c + 1) * 128, 1536:2048])
            for c in range(8):
                nc.sync.dma_start(xq_sb[c][:, 1536:2048], xq[c * 128:(c + 1) * 128, 1536:2048])

            def v_block(j):
                def emit():
                    vps = scr_pool.tile([128, 512], F32, tag="scr",
                                        name=f"vps{j}")
                    for c in range(8):
                        MM(vps[:],
                           xv_sb[c][:, j * 128:(j + 1) * 128],
                           wv_sb[c][:],
                           start=(c == 0), stop=(c == 7))
                    nc.vector.tensor_copy(v_sb[j][:], vps[:])
                return emit

            # pre-attention: just enough projections to start group 0
            k_chunk(0, 0)()
            q_chunk(0, 0)()
            q_chunk(0, 1)()

            # queue: V blocks + remaining K(0,*) during group 0; later K/Q
            # staggered ~1 item per kb ahead of their group deadlines
            for j in range(16):
                push(j + 3, v_block(j))
            push(3.5, k_chunk(0, 1))
            push(7.5, k_chunk(0, 2))
            push(11.2, k_chunk(0, 3))
            push(12.5, q_chunk(0, 2))
            push(14.5, q_chunk(0, 3))
            for m in range(1, 4):
                g_k = 2 * m * 16
                for n in range(4):
                    push(g_k + 4 * n - 6, k_chunk(m, n))
                for n in range(4):
                    push(g_k + 16 * (n // 2) - 8 + 4 * (n % 2), q_chunk(m, n))

            emit_group(0, 0, (0, 1), budget=1, horizon=4)

        # xv/wv SBUF freed; wo loads into that range
        wo_pool = ep(tc.tile_pool(name="wo", bufs=1))
        for p_ in range(4):
            wo_sb[p_] = wo_pool.tile([128, D], BF16, tag=f"wo{p_}", name=f"wo{p_}")
            nc.gpsimd.dma_start(wo_sb[p_][:], wo[p_ * 128:(p_ + 1) * 128, :])

        g_t = 16
        for (p, qqs) in GROUPS[1:]:
            budget = 2 if p == 3 else 1
            horizon = 48 if p == 3 else 20
            emit_group(g_t, p, qqs, budget=budget, horizon=horizon)
            g_t += 16

        # ---------- tail ----------
        drain_pvz(0, g_t)
        while pend_pe:
            pend_pe.pop(0)[2]()


def get_program():
    global _PROGRAM
    if _PROGRAM is None:
        _PROGRAM = _build_program()
    return _PROGRAM


def make_in_maps(Q_in, K_in, V_in, Wq, bq, Wk, bk, Wv, bv, Wo, bo):
    """Shard full inputs into 8 per-core input maps (bf16 pre-cast on host)."""
    scale = np.float32(1.0 / np.sqrt(DK))
    sel = np.zeros((2, 128), np.float32)
    sel[0, 0:64] = 1.0
    sel[1, 64:128] = 1.0
    sel = sel.astype(ml_dtypes.bfloat16)
    ones = np.ones((128, 1), ml_dtypes.bfloat16)

    def b16(a):
        return np.ascontiguousarray(np.asarray(a, np.float32).astype(ml_dtypes.bfloat16))

    xt = {}
    for b in range(B):
        xt[b] = (b16(np.asarray(Q_in[b], np.float32).T),
                 b16(np.asarray(K_in[b], np.float32).T),
                 b16(np.asarray(V_in[b], np.float32).T))

    in_maps = []
    for c in range(N_CORES):
        b, hh = c // 2, c % 2
        sl = slice(hh * FEAT, (hh + 1) * FEAT)
        in_maps.append({
            "xq_t": xt[b][0],
            "xk_t": xt[b][1],
            "xv_t": xt[b][2],
            "wq": b16(np.asarray(Wq, np.float32)[:, sl]),
            "wk": b16(np.asarray(Wk, np.float32)[:, sl] * scale),
            "wv": b16(np.asarray(Wv, np.float32)[:, sl]),
            "wo": b16(np.asarray(Wo, np.float32)[sl, :]),
            "bqk": np.ascontiguousarray(np.concatenate(
                [np.asarray(bq, np.float32)[sl].reshape(4, 128).T,
                 np.asarray(bk, np.float32)[sl].reshape(4, 128).T * scale],
                axis=1)),
            "sel": sel,
            "ones": ones,
        })
    return in_maps


def gather_output(results, Wo, bv, bo):
    """Combine per-core partial outputs into the full [B, S, D] output."""
    const = (np.asarray(bv, np.float32) @ np.asarray(Wo, np.float32)
             + np.asarray(bo, np.float32))
    out = np.empty((B, S, D), np.float32)
    for b in range(B):
        out[b] = (np.asarray(results[2 * b]["y"], np.float32)
                  + np.asarray(results[2 * b + 1]["y"], np.float32) + const)
    return out


def kernel(Q_in, K_in, V_in, Wq, bq, Wk, bk, Wv, bv, Wo, bo):
    nc = get_program()
    in_maps = make_in_maps(Q_in, K_in, V_in, Wq, bq, Wk, bk, Wv, bv, Wo, bo)
    res = bass_utils.run_bass_kernel_spmd(nc, in_maps, core_ids=list(range(N_CORES)))
    return gather_output(res.results, Wo, bv, bo)


# revision 63
# speedup vs baseline: 1.1344x; 1.1299x over previous
"""Multi-head attention (B=4, S=2048, D=1024, H=16) on 8 Trainium2 cores.

Sharding: core c -> (batch b = c//2, head-half hh = c%2).  Each core computes
8 heads of one batch: QKV projections with column-sliced weights, attention,
and a partial output projection with row-sliced Wo.  Host sums the two
partial outputs per batch and adds the constant bias terms.

The kernel is ACT(exp)-paced: 256 exp tiles of [128,1024] at ~1.34us each.
Everything else is scheduled to keep the scalar engine saturated:

  - attention runs in 9 groups: (p, qq-pair) for p=0..3 over head-pairs p,
    with the last p split into single-qq groups to shorten the output tail.
  - per key block kb: scores^T for both qq chunks (row-packed head pairs,
    2 concurrent N=512 matmuls per qq), exp on ACT, then (lagged by LAG kb)
    PV pairs (col-packed, concurrent) and Z row sums.
  - Z uses a [128,1] ones stationary -> M=1 matmuls, 4-way col-tiled at
    positions (0,0),(0,32),(0,64),(0,96): one concurrent round covers both
    heads x both qq chunks (~258ns).
  - softmax normalization: 1/Z via fast approx reciprocal on DVE (read
    straight from PSUM), cast to bf16, broadcast via rank-1 bf16 selector
    matmuls, applied to x^T with a DVE mul.
  - projections (V blocks, K/Q chunks), norm broadcasts and output pieces
    drain from a deadline-keyed work queue in the PE slack inside the
    ACT-paced loop.  Attention starts as soon as K(0,0)/Q(0,0..1) are
    projected (~15us) instead of after the full V projection.
  - input DMAs are split in token halves and spread over the sync /scalar /
    gpsimd /vector /tensor queues in deadline order; y is written in bf16
    on sync/gpsimd/vector (never scalar, which must stay free for exp).
"""
import numpy as np
import ml_dtypes

import concourse.tile as tile
from concourse import bacc, mybir
from concourse import bass_utils

F32 = mybir.dt.float32
BF16 = mybir.dt.bfloat16
EXP = mybir.ActivationFunctionType.Exp

B, S, D = 4, 2048, 1024
H = 16
DK = 64
FEAT = 512          # features per core (8 heads)
N_CORES = 8
LAG = 4             # PV/Z lag behind exp, in key blocks

_PROGRAM = None


def _build_program():
    nc = bacc.Bacc("TRN2", target_bir_lowering=False, debug=False,
                   enable_asserts=True, num_devices=N_CORES)

    xq = nc.dram_tensor("xq_t", [D, S], BF16, kind="ExternalInput").ap()
    xk = nc.dram_tensor("xk_t", [D, S], BF16, kind="ExternalInput").ap()
    xv = nc.dram_tensor("xv_t", [D, S], BF16, kind="ExternalInput").ap()
    wq = nc.dram_tensor("wq", [D, FEAT], BF16, kind="ExternalInput").ap()
    wk = nc.dram_tensor("wk", [D, FEAT], BF16, kind="ExternalInput").ap()
    wv = nc.dram_tensor("wv", [D, FEAT], BF16, kind="ExternalInput").ap()
    wo = nc.dram_tensor("wo", [FEAT, D], BF16, kind="ExternalInput").ap()
    bqk = nc.dram_tensor("bqk", [128, 8], F32, kind="ExternalInput").ap()
    sel = nc.dram_tensor("sel", [2, 128], BF16, kind="ExternalInput").ap()
    ones = nc.dram_tensor("ones", [128, 1], BF16, kind="ExternalInput").ap()
    y = nc.dram_tensor("y", [S, D], BF16, kind="ExternalOutput").ap()

    with tile.TileContext(nc) as tc:
        with nc.allow_low_precision(reason="bf16 matmul operand tiles"):
            _emit(nc, tc, xq, xk, xv, wq, wk, wv, wo, bqk, sel, ones, y)
    nc.compile()
    return nc


# attention groups: (p, qqs); last p split for a shorter output tail
GROUPS = [(0, (0, 1)), (0, (2, 3)),
          (1, (0, 1)), (1, (2, 3)),
          (2, (0, 1)), (2, (2, 3)),
          (3, (0, 1)), (3, (2,)), (3, (3,))]
ZROWS = ((0, 64), (32, 96))   # zp rows for qq index 0/1 within a group


def _emit(nc, tc, xq, xk, xv, wq, wk, wv, wo, bqk, sel, ones, y):
    from contextlib import ExitStack
    import bisect
    import itertools

    MM = nc.tensor.matmul

    with ExitStack() as ctx:
        ep = ctx.enter_context

        # ---------- persistent SBUF ----------
        qt_pool = ep(tc.tile_pool(name="qt", bufs=2))
        kt_pool = ep(tc.tile_pool(name="kt", bufs=2))
        v_pool = ep(tc.tile_pool(name="v", bufs=1))
        misc_pool = ep(tc.tile_pool(name="misc", bufs=1))
        xT_pool = ep(tc.tile_pool(name="xT", bufs=1))
        pt_pool = ep(tc.tile_pool(name="pt", bufs=2 * LAG + 2))
        rz_pool = ep(tc.tile_pool(name="rz", bufs=2))
        y_sb_pool = ep(tc.tile_pool(name="ysb", bufs=2))
        xk_pool = ep(tc.tile_pool(name="xk", bufs=1))
        wk_pool = ep(tc.tile_pool(name="wkp", bufs=1))
        xq_pool = ep(tc.tile_pool(name="xqp", bufs=1))
        wq_pool = ep(tc.tile_pool(name="wqp", bufs=1))

        # qt/kt rotate through 2 buffers: head-pair m's tiles are dead once
        # groups 2m/2m+1 finish, and m+2's projections pop much later
        _qt = {}
        _kt = {}

        def qt(m):
            if m not in _qt:
                _qt[m] = qt_pool.tile([128, S], BF16, tag="qt", name=f"qt{m}")
            return _qt[m]

        def kt(m):
            if m not in _kt:
                _kt[m] = kt_pool.tile([128, S], BF16, tag="kt", name=f"kt{m}")
            return _kt[m]
        v_sb = [v_pool.tile([128, FEAT], BF16, tag=f"v{k}", name=f"v{k}") for k in range(16)]
        xT = [xT_pool.tile([128, S], BF16, tag=f"xT{p}", name=f"xT{p}") for p in range(4)]

        bqk_sb = misc_pool.tile([128, 8], F32, tag="bqk")
        bq_sb = bqk_sb[:, 0:4]
        bk_sb = bqk_sb[:, 4:8]
        ones_sb = misc_pool.tile([128, 1], BF16, tag="ones")
        selA_sb = misc_pool.tile([1, 128], BF16, tag="selA")
        selB_sb = misc_pool.tile([1, 128], BF16, tag="selB")
        warm_sb = misc_pool.tile([1, 4], F32, tag="warm")

        xk_sb = [xk_pool.tile([128, S], BF16, tag=f"xk{c}", name=f"xk{c}") for c in range(8)]
        wk_sb = [wk_pool.tile([128, FEAT], BF16, tag=f"wk{c}", name=f"wk{c}") for c in range(8)]
        xq_sb = [xq_pool.tile([128, S], BF16, tag=f"xq{c}", name=f"xq{c}") for c in range(8)]
        wq_sb = [wq_pool.tile([128, FEAT], BF16, tag=f"wq{c}", name=f"wq{c}") for c in range(8)]

        # ---------- input DMAs, deadline-ordered ----------
        # A dma_start occupies its issuing engine queue for roughly the whole
        # transfer (~170GB/s per queue), so the layout is three parallel
        # streams with the scalar(ACT) queue freed early for the exps:
        #   sync:   consts, then xk/xv interleaved in token quarters
        #   scalar: warm-up exp (table preload), wq, xq q0/q1, then free
        #   gpsimd: wk, wv, xq q2/q3, wo
        nc.sync.dma_start(bqk_sb[:], bqk)
        nc.sync.dma_start(selA_sb[:], sel[0:1, :])
        nc.sync.dma_start(selB_sb[:], sel[1:2, :])
        nc.sync.dma_start(ones_sb[:], ones)
        nc.scalar.activation(warm_sb[:], bqk_sb[0:1, 0:4], EXP)
        for c in range(8):
            nc.gpsimd.dma_start(wk_sb[c][:], wk[c * 128:(c + 1) * 128, :])
        for c in range(8):
            nc.sync.dma_start(xk_sb[c][:, 0:512], xk[c * 128:(c + 1) * 128, 0:512])
        for c in range(8):
            nc.scalar.dma_start(wq_sb[c][:], wq[c * 128:(c + 1) * 128, :])
        for c in range(8):
            nc.scalar.dma_start(xq_sb[c][:, 0:512], xq[c * 128:(c + 1) * 128, 0:512])
        for c in range(8):
            nc.sync.dma_start(xq_sb[c][:, 512:1024], xq[c * 128:(c + 1) * 128, 512:1024])

        # ---------- PSUM pools ----------
        st_pool = ep(tc.tile_pool(name="st", bufs=2, space="PSUM"))
        pv_pool = ep(tc.tile_pool(name="pv", bufs=2, space="PSUM"))
        zp_pool = ep(tc.tile_pool(name="zp", bufs=1, space="PSUM"))
        scr_pool = ep(tc.tile_pool(name="scr", bufs=1, space="PSUM"))

        # ---------- work queue ----------
        pend_pe = []
        _tie = itertools.count()

        def push(key, emit):
            bisect.insort(pend_pe, (key, next(_tie), emit))

        def proj_chunk(m, n, w_sb, x_sb, dst, bias_sb, on_act=False):
            def emit():
                psq = scr_pool.tile([128, 512], F32, tag="scr",
                                    name=f"pj{id(dst)}{m}{n}")
                for c in range(8):
                    MM(psq[:],
                       w_sb[c][:, m * 128:(m + 1) * 128],
                       x_sb[c][:, n * 512:(n + 1) * 512],
                       start=(c == 0), stop=(c == 7))
                if on_act:
                    nc.scalar.activation(
                        dst(m)[:, n * 512:(n + 1) * 512], psq[:],
                        mybir.ActivationFunctionType.Identity,
                        bias=bias_sb[:, m:m + 1])
                else:
                    nc.vector.tensor_scalar_add(
                        dst(m)[:, n * 512:(n + 1) * 512], psq[:],
                        bias_sb[:, m:m + 1])
            return emit

        def k_chunk(m, n, on_act=False):
            return proj_chunk(m, n, wk_sb, xk_sb, kt, bk_sb, on_act)

        def q_chunk(m, n, on_act=False):
            return proj_chunk(m, n, wq_sb, xq_sb, qt, bq_sb, on_act)

        wo_sb = [None] * 4  # filled after the xv scope closes

        _dma_rr = itertools.count()

        def out_piece(qb, fo, pool_tag=None):
            def emit():
                pool, tag = pool_tag or (scr_pool, "scr")
                yp = pool.tile([128, 512], F32, tag=tag,
                               name=f"yp{qb}{fo}")
                for pp in range(4):
                    MM(yp[:],
                       xT[pp][:, qb * 128:(qb + 1) * 128],
                       wo_sb[pp][:, fo * 512:(fo + 1) * 512],
                       start=(pp == 0), stop=(pp == 3))
                ysb = y_sb_pool.tile([128, 512], BF16, tag="ysb")
                nc.vector.tensor_copy(ysb[:], yp[:])
                eng = (nc.sync, nc.gpsimd)[next(_dma_rr) % 2]
                eng.dma_start(
                    y[qb * 128:(qb + 1) * 128, fo * 512:(fo + 1) * 512],
                    ysb[:])
            return emit

        def norm_piece(p, qq, rzbA, rzbB, now_key):
            def emit():
                bc = scr_pool.tile([128, 512], F32, tag="scr",
                                   name=f"bc{p}{qq}")
                MM(bc[:], selA_sb[:], rzbA[:], start=True, stop=False)
                MM(bc[:], selB_sb[:], rzbB[:], start=False, stop=True)
                nc.vector.tensor_mul(xT[p][:, qq * 512:(qq + 1) * 512],
                                     xT[p][:, qq * 512:(qq + 1) * 512],
                                     bc[:])
                if p == 3:
                    # tail pieces (qq>=2) rotate scratch across the freed
                    # attention PSUM banks -> dense back-to-back matmuls
                    rot = ([(scr_pool, "scr")] if qq < 2 else
                           [(scr_pool, "scr"), (st_pool, "st"),
                            (pv_pool, "pv"), (zp_pool, "zp")])
                    for j, (qb, fo) in enumerate(
                            (qb, fo) for qb in range(4 * qq, 4 * qq + 4)
                            for fo in range(2)):
                        push(now_key + 2 + 0.5 * j,
                             out_piece(qb, fo, rot[j % len(rot)]))
            return emit

        # ---------- attention machinery ----------
        pend_pvz = []   # (p, kb, [(i, qq, pt)], pvs, zp, is_last, qqs)

        def emit_pvz(p, kb, parts, pvs, zp):
            for (i, qq, pt) in parts:
                MM(pvs[i][0:64, :],
                   v_sb[kb][:, p * 128:p * 128 + 64],
                   pt[:, 0:512],
                   tile_position=(0, 0),
                   start=(kb == 0), stop=(kb == 15))
                MM(pvs[i][64:128, :],
                   v_sb[kb][:, p * 128 + 64:p * 128 + 128],
                   pt[:, 512:1024],
                   tile_position=(0, 64),
                   start=(kb == 0), stop=(kb == 15),
                   skip_group_check=True)
            for (i, qq, pt) in parts:
                r0, r1 = ZROWS[i]
                MM(zp[r0:r0 + 1, :],
                   ones_sb[:],
                   pt[:, 0:512],
                   tile_position=(0, r0),
                   start=(kb == 0), stop=(kb == 15),
                   skip_group_check=True)
                MM(zp[r1:r1 + 1, :],
                   ones_sb[:],
                   pt[:, 512:1024],
                   tile_position=(0, r1),
                   start=(kb == 0), stop=(kb == 15),
                   skip_group_check=True)

        def finish_group(p, qqs, pvs, zp, now_key):
            # Ordered to release the PSUM WARs fastest: xT copies free the
            # pv banks, then z-row copies free the zp bank, then the
            # reciprocal/cast chain feeds the (delayed) norm broadcasts.
            for i, qq in enumerate(qqs):
                nc.vector.tensor_copy(xT[p][:, qq * 512:(qq + 1) * 512],
                                      pvs[i][:])
            zrows = []
            for i, qq in enumerate(qqs):
                r0, r1 = ZROWS[i]
                zA = rz_pool.tile([1, 512], F32, tag="zfa")
                zB = rz_pool.tile([1, 512], F32, tag="zfb")
                nc.vector.tensor_copy(zA[:], zp[r0:r0 + 1, :])
                nc.vector.tensor_copy(zB[:], zp[r1:r1 + 1, :])
                zrows.append((zA, zB))
            for i, qq in enumerate(qqs):
                zA, zB = zrows[i]
                rzbA = rz_pool.tile([1, 512], BF16, tag="rzba")
                rzbB = rz_pool.tile([1, 512], BF16, tag="rzbb")
                rzA = rz_pool.tile([1, 512], F32, tag="rzf", bufs=1)
                nc.vector.reciprocal_approx_fast(rzA[:], zA[:])
                nc.vector.tensor_copy(rzbA[:], rzA[:])
                rzB = rz_pool.tile([1, 512], F32, tag="rzf", bufs=1)
                nc.vector.reciprocal_approx_fast(rzB[:], zB[:])
                nc.vector.tensor_copy(rzbB[:], rzB[:])
                push(now_key + 5 + i, norm_piece(p, qq, rzbA, rzbB, now_key))

        def drain_pvz(n_keep, now_key):
            while len(pend_pvz) > n_keep:
                (p_, kb_, parts, pvs_, zp_, last, qqs_) = pend_pvz.pop(0)
                emit_pvz(p_, kb_, parts, pvs_, zp_)
                if last:
                    finish_group(p_, qqs_, pvs_, zp_, now_key)

        def pops(t, budget, horizon):
            while pend_pe and pend_pe[0][0] <= t + 1:
                pend_pe.pop(0)[2]()
            n = 0
            while n < budget and pend_pe and pend_pe[0][0] <= t + horizon:
                pend_pe.pop(0)[2]()
                n += 1

        def emit_group(g_t, p, qqs, budget, horizon):
            pvs = [pv_pool.tile([128, 512], F32, tag="pv",
                                name=f"pv{p}{qq}") for qq in qqs]
            zp = zp_pool.tile([128, 512], F32, tag="zp", name=f"zp{p}{qqs[0]}")
            for kb in range(16):
                t = g_t + kb
                parts = []
                for i, qq in enumerate(qqs):
                    st = st_pool.tile([128, 1024], F32, tag="st")
                    MM(st[:, 0:512],
                       kt(p)[0:64, kb * 128:(kb + 1) * 128],
                       qt(p)[0:64, qq * 512:(qq + 1) * 512],
                       start=True, stop=True)
                    MM(st[:, 512:1024],
                       kt(p)[64:128, kb * 128:(kb + 1) * 128],
                       qt(p)[64:128, qq * 512:(qq + 1) * 512],
                       start=True, stop=True)
                    pt = pt_pool.tile([128, 1024], BF16, tag="pt")
                    nc.scalar.activation(pt[:], st[:], EXP)
                    parts.append((i, qq, pt))
                pend_pvz.append((p, kb, parts, pvs, zp, kb == 15, qqs))
                # split the pops around the PV/Z drain: the attention matmuls
                # (no scratch-bank dependency) fill the scr WAR window between
                # consecutive queue items
                pops(t, 1, horizon)
                drain_pvz(LAG, t)
                if budget > 1:
                    pops(t, budget - 1, horizon)
            # group end: release any deadline-gated items, then drain the
            # PV/Z tail so the finish chain starts at the boundary; pop a
            # couple of filler items to keep the PE array warm (HAM) while
            # the DVE finish chain runs
            pops(g_t + 17, 0, 0)
            drain_pvz(0, g_t + 16)
            pops(g_t + 16, 2, 40)

        # ---------- head phase + group 0 (inside the xv/wv scope) ----------
        with tc.tile_pool(name="xv", bufs=1) as xv_pool, \
             tc.tile_pool(name="wvp", bufs=1) as wv_pool:
            xv_sb = [xv_pool.tile([128, S], BF16, tag=f"xv{c}", name=f"xv{c}")
                     for c in range(8)]
            wv_sb = [wv_pool.tile([128, FEAT], BF16, tag=f"wv{c}", name=f"wv{c}")
                     for c in range(8)]
            # remaining bulk in deadline order:
            #   sync:   xv q0, xk q2, xv q2, xq q2, xq q3
            #   gpsimd: wv, xk q1, xv q1, xk q3, xv q3 (then wo)
            for c in range(8):
                nc.gpsimd.dma_start(wv_sb[c][:], wv[c * 128:(c + 1) * 128, :])
            for c in range(8):
                nc.sync.dma_start(xv_sb[c][:, 0:512], xv[c * 128:(c + 1) * 128, 0:512])
            for c in range(8):
                nc.gpsimd.dma_start(xk_sb[c][:, 512:1024], xk[c * 128:(c + 1) * 128, 512:1024])
            for c in range(8):
                nc.sync.dma_start(xk_sb[c][:, 1024:1536], xk[c * 128:(c + 1) * 128, 1024:1536])
            for c in range(8):
                nc.gpsimd.dma_start(xv_sb[c][:, 512:1024], xv[c * 128:(c + 1) * 128, 512:1024])
            for c in range(8):
                nc.sync.dma_start(xv_sb[c][:, 1024:1536], xv[c * 128:(c + 1) * 128, 1024:1536])
            for c in range(8):
                nc.gpsimd.dma_start(xk_sb[c][:, 1536:2048], xk[c * 128:(c + 1) * 128, 1536:2048])
            for c in range(8):
                nc.sync.dma_start(xq_sb[c][:, 1024:1536], xq[c * 128:(c + 1) * 128, 1024:1536])
            for c in range(8):
                nc.gpsimd.dma_start(xv_sb[c][:, 1536:2048], xv[c * 128:(c + 1) * 128, 1536:2048])
            for c in range(8):
                nc.sync.dma_start(xq_sb[c][:, 1536:2048], xq[c * 128:(c + 1) * 128, 1536:2048])

            def v_block(j):
                def emit():
                    vps = scr_pool.tile([128, 512], F32, tag="scr",
                                        name=f"vps{j}")
                    for c in range(8):
                        MM(vps[:],
                           xv_sb[c][:, j * 128:(j + 1) * 128],
                           wv_sb[c][:],
                           start=(c == 0), stop=(c == 7))
                    nc.vector.tensor_copy(v_sb[j][:], vps[:])
                return emit

            # pre-attention: just enough projections to start group 0
            k_chunk(0, 0)()
            q_chunk(0, 0)()
            q_chunk(0, 1)()

            # queue: V blocks + remaining K(0,*) during group 0; later K/Q
            # staggered ~1 item per kb ahead of their group deadlines
            for j in range(16):
                push(j + 3, v_block(j))
            push(3.5, k_chunk(0, 1))
            push(7.5, k_chunk(0, 2))
            push(11.2, k_chunk(0, 3))
            push(12.5, q_chunk(0, 2))
            push(14.5, q_chunk(0, 3))
            for m in range(1, 4):
                g_k = 2 * m * 16
                for n in range(4):
                    push(g_k + 4 * n - 6, k_chunk(m, n))
                for n in range(4):
                    push(g_k + 16 * (n // 2) - 8 + 4 * (n % 2), q_chunk(m, n))

            emit_group(0, 0, (0, 1), budget=1, horizon=4)

        # xv/wv SBUF freed; wo loads into that range
        wo_pool = ep(tc.tile_pool(name="wo", bufs=1))
        for p_ in range(4):
            wo_sb[p_] = wo_pool.tile([128, D], BF16, tag=f"wo{p_}", name=f"wo{p_}")
            nc.gpsimd.dma_start(wo_sb[p_][:], wo[p_ * 128:(p_ + 1) * 128, :])

        g_t = 16
        for (p, qqs) in GROUPS[1:]:
            budget = 2 if p == 3 else 1
            horizon = 48 if p == 3 else 20
            emit_group(g_t, p, qqs, budget=budget, horizon=horizon)
            g_t += 16

        # ---------- tail ----------
        drain_pvz(0, g_t)
        while pend_pe:
            pend_pe.pop(0)[2]()


def get_program():
    global _PROGRAM
    if _PROGRAM is None:
        _PROGRAM = _build_program()
    return _PROGRAM


def make_in_maps(Q_in, K_in, V_in, Wq, bq, Wk, bk, Wv, bv, Wo, bo):
    """Shard full inputs into 8 per-core input maps (bf16 pre-cast on host)."""
    scale = np.float32(1.0 / np.sqrt(DK))
    sel = np.zeros((2, 128), np.float32)
    sel[0, 0:64] = 1.0
    sel[1, 64:128] = 1.0
    sel = sel.astype(ml_dtypes.bfloat16)
    ones = np.ones((128, 1), ml_dtypes.bfloat16)

    def b16(a):
        return np.ascontiguousarray(np.asarray(a, np.float32).astype(ml_dtypes.bfloat16))

    xt = {}
    for b in range(B):
        xt[b] = (b16(np.asarray(Q_in[b], np.float32).T),
                 b16(np.asarray(K_in[b], np.float32).T),
                 b16(np.asarray(V_in[b], np.float32).T))

    in_maps = []
    for c in range(N_CORES):
        b, hh = c // 2, c % 2
        sl = slice(hh * FEAT, (hh + 1) * FEAT)
        in_maps.append({
            "xq_t": xt[b][0],
            "xk_t": xt[b][1],
            "xv_t": xt[b][2],
            "wq": b16(np.asarray(Wq, np.float32)[:, sl]),
            "wk": b16(np.asarray(Wk, np.float32)[:, sl] * scale),
            "wv": b16(np.asarray(Wv, np.float32)[:, sl]),
            "wo": b16(np.asarray(Wo, np.float32)[sl, :]),
            "bqk": np.ascontiguousarray(np.concatenate(
                [np.asarray(bq, np.float32)[sl].reshape(4, 128).T,
                 np.asarray(bk, np.float32)[sl].reshape(4, 128).T * scale],
                axis=1)),
            "sel": sel,
            "ones": ones,
        })
    return in_maps


def gather_output(results, Wo, bv, bo):
    """Combine per-core partial outputs into the full [B, S, D] output."""
    const = (np.asarray(bv, np.float32) @ np.asarray(Wo, np.float32)
             + np.asarray(bo, np.float32))
    out = np.empty((B, S, D), np.float32)
    for b in range(B):
        out[b] = (np.asarray(results[2 * b]["y"], np.float32)
                  + np.asarray(results[2 * b + 1]["y"], np.float32) + const)
    return out


def kernel(Q_in, K_in, V_in, Wq, bq, Wk, bk, Wv, bv, Wo, bo):
    nc = get_program()
    in_maps = make_in_maps(Q_in, K_in, V_in, Wq, bq, Wk, bk, Wv, bv, Wo, bo)
    res = bass_utils.run_bass_kernel_spmd(nc, in_maps, core_ids=list(range(N_CORES)))
    return gather_output(res.results, Wo, bv, bo)


# revision 64
# speedup vs baseline: 1.1550x; 1.0182x over previous
"""Multi-head attention (B=4, S=2048, D=1024, H=16) on 8 Trainium2 cores.

Sharding: core c -> (batch b = c//2, head-half hh = c%2).  Each core computes
8 heads of one batch: QKV projections with column-sliced weights, attention,
and a partial output projection with row-sliced Wo.  Host sums the two
partial outputs per batch and adds the constant bias terms.

The kernel is ACT(exp)-paced: 256 exp tiles of [128,1024] at ~1.34us each.
Everything else is scheduled to keep the scalar engine saturated:

  - attention runs in 9 groups: (p, qq-pair) for p=0..3 over head-pairs p,
    with the last p split into single-qq groups to shorten the output tail.
  - per key block kb: scores^T for both qq chunks (row-packed head pairs,
    2 concurrent N=512 matmuls per qq), exp on ACT, then (lagged by LAG kb)
    PV pairs (col-packed, concurrent) and Z row sums.
  - Z uses a [128,1] ones stationary -> M=1 matmuls, 4-way col-tiled at
    positions (0,0),(0,32),(0,64),(0,96): one concurrent round covers both
    heads x both qq chunks (~258ns).
  - softmax normalization: 1/Z via fast approx reciprocal on DVE (read
    straight from PSUM), cast to bf16, broadcast via rank-1 bf16 selector
    matmuls, applied to x^T with a DVE mul.
  - projections (V blocks, K/Q chunks), norm broadcasts and output pieces
    drain from a deadline-keyed work queue in the PE slack inside the
    ACT-paced loop.  Attention starts as soon as K(0,0)/Q(0,0..1) are
    projected (~15us) instead of after the full V projection.
  - input DMAs are split in token halves and spread over the sync /scalar /
    gpsimd /vector /tensor queues in deadline order; y is written in bf16
    on sync/gpsimd/vector (never scalar, which must stay free for exp).
"""
import numpy as np
import ml_dtypes

import concourse.tile as tile
from concourse import bacc, mybir
from concourse import bass_utils

F32 = mybir.dt.float32
BF16 = mybir.dt.bfloat16
EXP = mybir.ActivationFunctionType.Exp

B, S, D = 4, 2048, 1024
H = 16
DK = 64
FEAT = 512          # features per core (8 heads)
N_CORES = 8
LAG = 4             # PV/Z lag behind exp, in key blocks

_PROGRAM = None


def _build_program():
    nc = bacc.Bacc("TRN2", target_bir_lowering=False, debug=False,
                   enable_asserts=True, num_devices=N_CORES)

    xq = nc.dram_tensor("xq_t", [D, S], BF16, kind="ExternalInput").ap()
    xk = nc.dram_tensor("xk_t", [D, S], BF16, kind="ExternalInput").ap()
    xv = nc.dram_tensor("xv_t", [D, S], BF16, kind="ExternalInput").ap()
    wq = nc.dram_tensor("wq", [D, FEAT], BF16, kind="ExternalInput").ap()
    wk = nc.dram_tensor("wk", [D, FEAT], BF16, kind="ExternalInput").ap()
    wv = nc.dram_tensor("wv", [D, FEAT], BF16, kind="ExternalInput").ap()
    wo = nc.dram_tensor("wo", [FEAT, D], BF16, kind="ExternalInput").ap()
    bqk = nc.dram_tensor("bqk", [128, 8], F32, kind="ExternalInput").ap()
    sel = nc.dram_tensor("sel", [2, 128], BF16, kind="ExternalInput").ap()
    ones = nc.dram_tensor("ones", [128, 1], BF16, kind="ExternalInput").ap()
    y = nc.dram_tensor("y", [S, D], BF16, kind="ExternalOutput").ap()

    with tile.TileContext(nc) as tc:
        with nc.allow_low_precision(reason="bf16 matmul operand tiles"):
            _emit(nc, tc, xq, xk, xv, wq, wk, wv, wo, bqk, sel, ones, y)
    nc.compile()
    return nc


# attention groups: (p, qqs); last p split for a shorter output tail
GROUPS = [(0, (0, 1)), (0, (2, 3)),
          (1, (0, 1)), (1, (2, 3)),
          (2, (0, 1)), (2, (2, 3)),
          (3, (0, 1)), (3, (2,)), (3, (3,))]
ZROWS = ((0, 64), (32, 96))   # zp rows for qq index 0/1 within a group


def _emit(nc, tc, xq, xk, xv, wq, wk, wv, wo, bqk, sel, ones, y):
    from contextlib import ExitStack
    import bisect
    import itertools

    MM = nc.tensor.matmul

    with ExitStack() as ctx:
        ep = ctx.enter_context

        # ---------- persistent SBUF ----------
        qt_pool = ep(tc.tile_pool(name="qt", bufs=2))
        kt_pool = ep(tc.tile_pool(name="kt", bufs=2))
        v_pool = ep(tc.tile_pool(name="v", bufs=1))
        misc_pool = ep(tc.tile_pool(name="misc", bufs=1))
        xT_pool = ep(tc.tile_pool(name="xT", bufs=1))
        pt_pool = ep(tc.tile_pool(name="pt", bufs=2 * LAG + 2))
        rz_pool = ep(tc.tile_pool(name="rz", bufs=2))
        y_sb_pool = ep(tc.tile_pool(name="ysb", bufs=2))
        xk_pool = ep(tc.tile_pool(name="xk", bufs=1))
        wk_pool = ep(tc.tile_pool(name="wkp", bufs=1))
        xq_pool = ep(tc.tile_pool(name="xqp", bufs=1))
        wq_pool = ep(tc.tile_pool(name="wqp", bufs=1))

        # qt/kt rotate through 2 buffers: head-pair m's tiles are dead once
        # groups 2m/2m+1 finish, and m+2's projections pop much later
        _qt = {}
        _kt = {}

        def qt(m):
            if m not in _qt:
                _qt[m] = qt_pool.tile([128, S], BF16, tag="qt", name=f"qt{m}")
            return _qt[m]

        def kt(m):
            if m not in _kt:
                _kt[m] = kt_pool.tile([128, S], BF16, tag="kt", name=f"kt{m}")
            return _kt[m]
        v_sb = [v_pool.tile([128, FEAT], BF16, tag=f"v{k}", name=f"v{k}") for k in range(16)]
        xT = [xT_pool.tile([128, S], BF16, tag=f"xT{p}", name=f"xT{p}") for p in range(4)]

        bqk_sb = misc_pool.tile([128, 8], F32, tag="bqk")
        bq_sb = bqk_sb[:, 0:4]
        bk_sb = bqk_sb[:, 4:8]
        ones_sb = misc_pool.tile([128, 1], BF16, tag="ones")
        selA_sb = misc_pool.tile([1, 128], BF16, tag="selA")
        selB_sb = misc_pool.tile([1, 128], BF16, tag="selB")
        warm_sb = misc_pool.tile([1, 4], F32, tag="warm")

        xk_sb = [xk_pool.tile([128, S], BF16, tag=f"xk{c}", name=f"xk{c}") for c in range(8)]
        wk_sb = [wk_pool.tile([128, FEAT], BF16, tag=f"wk{c}", name=f"wk{c}") for c in range(8)]
        xq_sb = [xq_pool.tile([128, S], BF16, tag=f"xq{c}", name=f"xq{c}") for c in range(8)]
        wq_sb = [wq_pool.tile([128, FEAT], BF16, tag=f"wq{c}", name=f"wq{c}") for c in range(8)]

        # ---------- input DMAs, deadline-ordered ----------
        # A dma_start occupies its issuing engine queue for roughly the whole
        # transfer (~170GB/s per queue), so the layout is three parallel
        # streams with the scalar(ACT) queue freed early for the exps:
        #   sync:   consts, then xk/xv interleaved in token quarters
        #   scalar: warm-up exp (table preload), wq, xq q0/q1, then free
        #   gpsimd: wk, wv, xq q2/q3, wo
        nc.sync.dma_start(bqk_sb[:], bqk)
        nc.sync.dma_start(selA_sb[:], sel[0:1, :])
        nc.sync.dma_start(selB_sb[:], sel[1:2, :])
        nc.sync.dma_start(ones_sb[:], ones)
        nc.scalar.activation(warm_sb[:], bqk_sb[0:1, 0:4], EXP)
        for c in range(8):
            nc.gpsimd.dma_start(wk_sb[c][:], wk[c * 128:(c + 1) * 128, :])
        for c in range(8):
            nc.sync.dma_start(xk_sb[c][:, 0:512], xk[c * 128:(c + 1) * 128, 0:512])
        for c in range(8):
            nc.scalar.dma_start(wq_sb[c][:], wq[c * 128:(c + 1) * 128, :])
        for c in range(8):
            nc.scalar.dma_start(xq_sb[c][:, 0:512], xq[c * 128:(c + 1) * 128, 0:512])
        for c in range(8):
            nc.sync.dma_start(xq_sb[c][:, 512:1024], xq[c * 128:(c + 1) * 128, 512:1024])

        # ---------- PSUM pools ----------
        st_pool = ep(tc.tile_pool(name="st", bufs=2, space="PSUM"))
        pv_pool = ep(tc.tile_pool(name="pv", bufs=2, space="PSUM"))
        zp_pool = ep(tc.tile_pool(name="zp", bufs=1, space="PSUM"))
        scr_pool = ep(tc.tile_pool(name="scr", bufs=1, space="PSUM"))

        # ---------- work queue ----------
        pend_pe = []
        _tie = itertools.count()

        def push(key, emit):
            bisect.insort(pend_pe, (key, next(_tie), emit))

        def proj_chunk(m, n, w_sb, x_sb, dst, bias_sb, on_act=False):
            def emit():
                psq = scr_pool.tile([128, 512], F32, tag="scr",
                                    name=f"pj{id(dst)}{m}{n}")
                for c in range(8):
                    MM(psq[:],
                       w_sb[c][:, m * 128:(m + 1) * 128],
                       x_sb[c][:, n * 512:(n + 1) * 512],
                       start=(c == 0), stop=(c == 7))
                if on_act:
                    nc.scalar.activation(
                        dst(m)[:, n * 512:(n + 1) * 512], psq[:],
                        mybir.ActivationFunctionType.Identity,
                        bias=bias_sb[:, m:m + 1])
                else:
                    nc.vector.tensor_scalar_add(
                        dst(m)[:, n * 512:(n + 1) * 512], psq[:],
                        bias_sb[:, m:m + 1])
            return emit

        def k_chunk(m, n, on_act=False):
            return proj_chunk(m, n, wk_sb, xk_sb, kt, bk_sb, on_act)

        def q_chunk(m, n, on_act=False):
            return proj_chunk(m, n, wq_sb, xq_sb, qt, bq_sb, on_act)

        wo_sb = [None] * 4  # filled after the xv scope closes

        _dma_rr = itertools.count()

        def out_piece(qb, fo, pool_tag=None):
            def emit():
                pool, tag = pool_tag or (scr_pool, "scr")
                yp = pool.tile([128, 512], F32, tag=tag,
                               name=f"yp{qb}{fo}")
                for pp in range(4):
                    MM(yp[:],
                       xT[pp][:, qb * 128:(qb + 1) * 128],
                       wo_sb[pp][:, fo * 512:(fo + 1) * 512],
                       start=(pp == 0), stop=(pp == 3))
                ysb = y_sb_pool.tile([128, 512], BF16, tag="ysb")
                nc.vector.tensor_copy(ysb[:], yp[:])
                eng = (nc.sync, nc.gpsimd)[next(_dma_rr) % 2]
                eng.dma_start(
                    y[qb * 128:(qb + 1) * 128, fo * 512:(fo + 1) * 512],
                    ysb[:])
            return emit

        def norm_piece(p, qq, rzbA, rzbB, now_key):
            def emit():
                bc = scr_pool.tile([128, 512], F32, tag="scr",
                                   name=f"bc{p}{qq}")
                MM(bc[:], selA_sb[:], rzbA[:], start=True, stop=False)
                MM(bc[:], selB_sb[:], rzbB[:], start=False, stop=True)
                nc.vector.tensor_mul(xT[p][:, qq * 512:(qq + 1) * 512],
                                     xT[p][:, qq * 512:(qq + 1) * 512],
                                     bc[:])
                if p == 3:
                    # tail pieces (qq>=2) rotate scratch across the freed
                    # attention PSUM banks -> dense back-to-back matmuls
                    rot = ([(scr_pool, "scr")] if qq < 2 else
                           [(scr_pool, "scr"), (st_pool, "st"),
                            (pv_pool, "pv"), (zp_pool, "zp")])
                    for j, (qb, fo) in enumerate(
                            (qb, fo) for qb in range(4 * qq, 4 * qq + 4)
                            for fo in range(2)):
                        push(now_key + 2 + 0.5 * j,
                             out_piece(qb, fo, rot[j % len(rot)]))
            return emit

        # ---------- attention machinery ----------
        pend_pvz = []   # (p, kb, [(i, qq, pt)], pvs, zp, is_last, qqs)

        def emit_pvz(p, kb, parts, pvs, zp):
            for (i, qq, pt) in parts:
                MM(pvs[i][0:64, :],
                   v_sb[kb][:, p * 128:p * 128 + 64],
                   pt[:, 0:512],
                   tile_position=(0, 0),
                   start=(kb == 0), stop=(kb == 15))
                MM(pvs[i][64:128, :],
                   v_sb[kb][:, p * 128 + 64:p * 128 + 128],
                   pt[:, 512:1024],
                   tile_position=(0, 64),
                   start=(kb == 0), stop=(kb == 15),
                   skip_group_check=True)
            for (i, qq, pt) in parts:
                r0, r1 = ZROWS[i]
                MM(zp[r0:r0 + 1, :],
                   ones_sb[:],
                   pt[:, 0:512],
                   tile_position=(0, r0),
                   start=(kb == 0), stop=(kb == 15),
                   skip_group_check=True)
                MM(zp[r1:r1 + 1, :],
                   ones_sb[:],
                   pt[:, 512:1024],
                   tile_position=(0, r1),
                   start=(kb == 0), stop=(kb == 15),
                   skip_group_check=True)

        def finish_group(p, qqs, pvs, zp, now_key):
            # Ordered to release the PSUM WARs fastest: xT copies free the
            # pv banks, then z-row copies free the zp bank, then the
            # reciprocal/cast chain feeds the (delayed) norm broadcasts.
            for i, qq in enumerate(qqs):
                nc.vector.tensor_copy(xT[p][:, qq * 512:(qq + 1) * 512],
                                      pvs[i][:])
            zrows = []
            for i, qq in enumerate(qqs):
                r0, r1 = ZROWS[i]
                zA = rz_pool.tile([1, 512], F32, tag="zfa")
                zB = rz_pool.tile([1, 512], F32, tag="zfb")
                nc.vector.tensor_copy(zA[:], zp[r0:r0 + 1, :])
                nc.vector.tensor_copy(zB[:], zp[r1:r1 + 1, :])
                zrows.append((zA, zB))
            for i, qq in enumerate(qqs):
                zA, zB = zrows[i]
                rzbA = rz_pool.tile([1, 512], BF16, tag="rzba")
                rzbB = rz_pool.tile([1, 512], BF16, tag="rzbb")
                rzA = rz_pool.tile([1, 512], F32, tag="rzf", bufs=1)
                nc.vector.reciprocal_approx_fast(rzA[:], zA[:])
                nc.vector.tensor_copy(rzbA[:], rzA[:])
                rzB = rz_pool.tile([1, 512], F32, tag="rzf", bufs=1)
                nc.vector.reciprocal_approx_fast(rzB[:], zB[:])
                nc.vector.tensor_copy(rzbB[:], rzB[:])
                push(now_key + 5 + i, norm_piece(p, qq, rzbA, rzbB, now_key))

        def drain_pvz(n_keep, now_key):
            while len(pend_pvz) > n_keep:
                (p_, kb_, parts, pvs_, zp_, last, qqs_) = pend_pvz.pop(0)
                emit_pvz(p_, kb_, parts, pvs_, zp_)
                if last:
                    finish_group(p_, qqs_, pvs_, zp_, now_key)

        def pops(t, budget, horizon):
            while pend_pe and pend_pe[0][0] <= t + 1:
                pend_pe.pop(0)[2]()
            n = 0
            while n < budget and pend_pe and pend_pe[0][0] <= t + horizon:
                pend_pe.pop(0)[2]()
                n += 1

        def emit_group(g_t, p, qqs, budget, horizon):
            pvs = [pv_pool.tile([128, 512], F32, tag="pv",
                                name=f"pv{p}{qq}") for qq in qqs]
            zp = zp_pool.tile([128, 512], F32, tag="zp", name=f"zp{p}{qqs[0]}")
            for kb in range(16):
                t = g_t + kb
                parts = []
                for i, qq in enumerate(qqs):
                    st = st_pool.tile([128, 1024], F32, tag="st")
                    MM(st[:, 0:512],
                       kt(p)[0:64, kb * 128:(kb + 1) * 128],
                       qt(p)[0:64, qq * 512:(qq + 1) * 512],
                       start=True, stop=True)
                    MM(st[:, 512:1024],
                       kt(p)[64:128, kb * 128:(kb + 1) * 128],
                       qt(p)[64:128, qq * 512:(qq + 1) * 512],
                       start=True, stop=True)
                    pt = pt_pool.tile([128, 1024], BF16, tag="pt")
                    nc.scalar.activation(pt[:], st[:], EXP)
                    parts.append((i, qq, pt))
                pend_pvz.append((p, kb, parts, pvs, zp, kb == 15, qqs))
                # split the pops around the PV/Z drain: the attention matmuls
                # (no scratch-bank dependency) fill the scr WAR window between
                # consecutive queue items
                pops(t, 1, horizon)
                drain_pvz(LAG, t)
                if budget > 1:
                    pops(t, budget - 1, horizon)
            # group end: release any deadline-gated items, then drain the
            # PV/Z tail so the finish chain starts at the boundary; pop a
            # couple of filler items to keep the PE array warm (HAM) while
            # the DVE finish chain runs
            pops(g_t + 17, 0, 0)
            drain_pvz(0, g_t + 16)
            pops(g_t + 16, 3, 40)

        # ---------- head phase + group 0 (inside the xv/wv scope) ----------
        with tc.tile_pool(name="xv", bufs=1) as xv_pool, \
             tc.tile_pool(name="wvp", bufs=1) as wv_pool:
            xv_sb = [xv_pool.tile([128, S], BF16, tag=f"xv{c}", name=f"xv{c}")
                     for c in range(8)]
            wv_sb = [wv_pool.tile([128, FEAT], BF16, tag=f"wv{c}", name=f"wv{c}")
                     for c in range(8)]
            # remaining bulk in deadline order:
            #   sync:   xv q0, xk q2, xv q2, xq q2, xq q3
            #   gpsimd: wv, xk q1, xv q1, xk q3, xv q3 (then wo)
            for c in range(8):
                nc.gpsimd.dma_start(wv_sb[c][:], wv[c * 128:(c + 1) * 128, :])
            for c in range(8):
                nc.sync.dma_start(xv_sb[c][:, 0:512], xv[c * 128:(c + 1) * 128, 0:512])
            for c in range(8):
                nc.gpsimd.dma_start(xk_sb[c][:, 512:1024], xk[c * 128:(c + 1) * 128, 512:1024])
            for c in range(8):
                nc.sync.dma_start(xk_sb[c][:, 1024:1536], xk[c * 128:(c + 1) * 128, 1024:1536])
            for c in range(8):
                nc.gpsimd.dma_start(xv_sb[c][:, 512:1024], xv[c * 128:(c + 1) * 128, 512:1024])
            for c in range(8):
                nc.sync.dma_start(xv_sb[c][:, 1024:1536], xv[c * 128:(c + 1) * 128, 1024:1536])
            for c in range(8):
                nc.gpsimd.dma_start(xk_sb[c][:, 1536:2048], xk[c * 128:(c + 1) * 128, 1536:2048])
            for c in range(8):
                nc.sync.dma_start(xq_sb[c][:, 1024:1536], xq[c * 128:(c + 1) * 128, 1024:1536])
            for c in range(8):
                nc.gpsimd.dma_start(xv_sb[c][:, 1536:2048], xv[c * 128:(c + 1) * 128, 1536:2048])
            for c in range(8):
                nc.sync.dma_start(xq_sb[c][:, 1536:2048], xq[c * 128:(c + 1) * 128, 1536:2048])

            def v_block(j):
                def emit():
                    vps = scr_pool.tile([128, 512], F32, tag="scr",
                                        name=f"vps{j}")
                    for c in range(8):
                        MM(vps[:],
                           xv_sb[c][:, j * 128:(j + 1) * 128],
                           wv_sb[c][:],
                           start=(c == 0), stop=(c == 7))
                    nc.vector.tensor_copy(v_sb[j][:], vps[:])
                return emit

            # pre-attention: just enough projections to start group 0
            k_chunk(0, 0)()
            q_chunk(0, 0)()
            q_chunk(0, 1)()

            # queue: V blocks + remaining K(0,*) during group 0; later K/Q
            # staggered ~1 item per kb ahead of their group deadlines
            for j in range(16):
                push(j + 3, v_block(j))
            push(3.5, k_chunk(0, 1))
            push(7.5, k_chunk(0, 2))
            push(11.2, k_chunk(0, 3))
            push(12.5, q_chunk(0, 2))
            push(14.5, q_chunk(0, 3))
            for m in range(1, 4):
                g_k = 2 * m * 16
                for n in range(4):
                    push(g_k + 4 * n - 6, k_chunk(m, n))
                for n in range(4):
                    push(g_k + 16 * (n // 2) - 8 + 4 * (n % 2), q_chunk(m, n))

            emit_group(0, 0, (0, 1), budget=1, horizon=4)

        # xv/wv SBUF freed; wo loads into that range
        wo_pool = ep(tc.tile_pool(name="wo", bufs=1))
        for p_ in range(4):
            wo_sb[p_] = wo_pool.tile([128, D], BF16, tag=f"wo{p_}", name=f"wo{p_}")
            nc.gpsimd.dma_start(wo_sb[p_][:], wo[p_ * 128:(p_ + 1) * 128, :])

        g_t = 16
        for (p, qqs) in GROUPS[1:]:
            budget = 2 if p == 3 else 1
            horizon = 48 if p == 3 else 20
            emit_group(g_t, p, qqs, budget=budget, horizon=horizon)
            g_t += 16

        # ---------- tail ----------
        drain_pvz(0, g_t)
        while pend_pe:
            pend_pe.pop(0)[2]()


def get_program():
    global _PROGRAM
    if _PROGRAM is None:
        _PROGRAM = _build_program()
    return _PROGRAM


def make_in_maps(Q_in, K_in, V_in, Wq, bq, Wk, bk, Wv, bv, Wo, bo):
    """Shard full inputs into 8 per-core input maps (bf16 pre-cast on host)."""
    scale = np.float32(1.0 / np.sqrt(DK))
    sel = np.zeros((2, 128), np.float32)
    sel[0, 0:64] = 1.0
    sel[1, 64:128] = 1.0
    sel = sel.astype(ml_dtypes.bfloat16)
    ones = np.ones((128, 1), ml_dtypes.bfloat16)

    def b16(a):
        return np.ascontiguousarray(np.asarray(a, np.float32).astype(ml_dtypes.bfloat16))

    xt = {}
    for b in range(B):
        xt[b] = (b16(np.asarray(Q_in[b], np.float32).T),
                 b16(np.asarray(K_in[b], np.float32).T),
                 b16(np.asarray(V_in[b], np.float32).T))

    in_maps = []
    for c in range(N_CORES):
        b, hh = c // 2, c % 2
        sl = slice(hh * FEAT, (hh + 1) * FEAT)
        in_maps.append({
            "xq_t": xt[b][0],
            "xk_t": xt[b][1],
            "xv_t": xt[b][2],
            "wq": b16(np.asarray(Wq, np.float32)[:, sl]),
            "wk": b16(np.asarray(Wk, np.float32)[:, sl] * scale),
            "wv": b16(np.asarray(Wv, np.float32)[:, sl]),
            "wo": b16(np.asarray(Wo, np.float32)[sl, :]),
            "bqk": np.ascontiguousarray(np.concatenate(
                [np.asarray(bq, np.float32)[sl].reshape(4, 128).T,
                 np.asarray(bk, np.float32)[sl].reshape(4, 128).T * scale],
                axis=1)),
            "sel": sel,
            "ones": ones,
        })
    return in_maps


def gather_output(results, Wo, bv, bo):
    """Combine per-core partial outputs into the full [B, S, D] output."""
    const = (np.asarray(bv, np.float32) @ np.asarray(Wo, np.float32)
             + np.asarray(bo, np.float32))
    out = np.empty((B, S, D), np.float32)
    for b in range(B):
        out[b] = (np.asarray(results[2 * b]["y"], np.float32)
                  + np.asarray(results[2 * b + 1]["y"], np.float32) + const)
    return out


def kernel(Q_in, K_in, V_in, Wq, bq, Wk, bk, Wv, bv, Wo, bo):
    nc = get_program()
    in_maps = make_in_maps(Q_in, K_in, V_in, Wq, bq, Wk, bk, Wv, bv, Wo, bo)
    res = bass_utils.run_bass_kernel_spmd(nc, in_maps, core_ids=list(range(N_CORES)))
    return gather_output(res.results, Wo, bv, bo)
